# revision 14
# baseline (speedup 1.0000x reference)
"""Bass/Trainium2 kernel for a single-head causal decoder attention head.

Reference computation (fp32):
    k = x @ Wk; q = x @ Wq; v = x @ Wv            # [B,T,H]
    att = softmax(causal(q k^T / sqrt(H)))        # [B,T,T]
    out = att @ v                                 # [B,T,H]
with B=4, T=4096, C=1024, H=128.

Sharding: 8 cores = 4 batches x 2 query-interleave lanes (j in {0,1}).
Core (b, j) handles q-groups [(2i+j)*512, (2i+j+1)*512) for i in 0..3 and
runs a *uniform* kv-span schedule {1024, 2048, 3072, 4096} for groups
0..3, so all 8 cores execute the same instruction stream (SPMD, one NEFF)
while per-core DRAM data (x^T slices, q-column gather, mask stack) makes
the math come out right.  Causality beyond each group's true span is
enforced by additive -30000 masks on the last 8 kv chunks of each group.

Dataflow (per core, transposed land so no on-chip transposes are needed):
    KT [H, T]   = Wk^T x^T        (8 c-chunk matmuls per 512 kv cols)
    QT [H, 2048]= Wq^T xq^T
    V  [kv, H]  (32 blocks)       (lhsT = x^T chunk, rhs = Wv chunk)
    per q-group, per kv chunk c:
        S^T  = KT_c^T QT_g                 (PSUM [128kv, 512q])
        S^T += mask (last 8 chunks, DVE)
        P^T  = exp(S^T / sqrt(H))          (ACT, bf16 -> SBUF)
        outT += V_c^T P^T                  (PSUM [128H, 512q], accumulated)
        sums += ones^T P^T                 (PSUM [1, 512q], accumulated)
    outT / sums -> DRAM  (reciprocal + partition-broadcast + multiply)
"""

import sys

sys.path.insert(0, "/opt/trn_rl_repo")

import numpy as np
import ml_dtypes

import concourse.bass as bass
import concourse.mybir as mybir
import concourse.tile as tile
from concourse import bacc
from concourse.alu_op_type import AluOpType
from concourse.masks import make_identity
from concourse.bass_utils import run_bass_kernel_spmd

B, T, C, H = 4, 4096, 1024, 128
NCORES = 8
QG = 512                      # q-group width
NG = 4                        # q-groups per core
SPANS = [1024, 2048, 3072, 4096]  # uniform kv span per group index
CB = C // 128                 # 8 contraction chunks
TGRP = T // QG                # 8 kv col-groups for projections
SCALE = float(H) ** -0.5
MASKVAL = -30000.0

BF16 = mybir.dt.bfloat16
F32 = mybir.dt.float32
NPBF16 = ml_dtypes.bfloat16


def _build_program():
    nc = bacc.Bacc("TRN2", target_bir_lowering=False, debug=False)

    xt = nc.dram_tensor("xt", [C, T], BF16, kind="ExternalInput").ap()
    xtq = nc.dram_tensor("xtq", [C, NG * QG], BF16, kind="ExternalInput").ap()
    wk = nc.dram_tensor("wk", [C, H], BF16, kind="ExternalInput").ap()
    wq = nc.dram_tensor("wq", [C, H], BF16, kind="ExternalInput").ap()
    wv = nc.dram_tensor("wv", [C, H], BF16, kind="ExternalInput").ap()
    msk = nc.dram_tensor("msk", [8, 128, QG], BF16, kind="ExternalInput").ap()
    outT = nc.dram_tensor("outT", [H, NG * QG], F32, kind="ExternalOutput").ap()

    with tile.TileContext(nc) as tc:
        with (
            tc.tile_pool(name="const", bufs=1) as constp,
            tc.tile_pool(name="kvq", bufs=1) as kvqp,
            tc.tile_pool(name="xin", bufs=2) as xinp,
            tc.tile_pool(name="attb", bufs=4) as attp,
            tc.tile_pool(name="epi", bufs=2) as epip,
        ):
            # --- persistent SBUF tensors ---
            wks = constp.tile([128, CB * H], BF16, tag="wks")
            wqs = constp.tile([128, CB * H], BF16, tag="wqs")
            wvs = constp.tile([128, CB * H], BF16, tag="wvs")
            for eng, ws, w in (
                (nc.scalar, wks, wk), (nc.scalar, wqs, wq), (nc.gpsimd, wvs, wv)
            ):
                eng.dma_start(
                    ws.rearrange("p (c h) -> p c h", c=CB),
                    w.rearrange("(c p) h -> p c h", p=128),
                )
            masks = constp.tile([128, 8 * QG], BF16, tag="masks")
            ident = constp.tile([128, 128], BF16, tag="ident")
            make_identity(nc, ident)

            KT = kvqp.tile([128, T], BF16, tag="KT")
            VT = kvqp.tile([128, T], BF16, tag="VT")
            NKV = T // 128
            VVa = kvqp.tile([128, NKV * 65], BF16, tag="VVa")
            VVb = kvqp.tile([128, NKV * 64], BF16, tag="VVb")
            QT = kvqp.tile([128, NG * QG], BF16, tag="QT")
            # ones column of every VVa chunk
            nc.vector.memset(
                VVa.rearrange("p (c w) -> p c w", w=65)[:, :, 64:65], 1.0
            )

            # --- phase 1: projections ---
            with tc.tile_pool(name="pp", bufs=2, space="PSUM") as ppool:
                xtr = xt.rearrange("(c p) t -> p c t", p=128)
                for tg in range(TGRP):
                    xg = xinp.tile([128, CB * QG], BF16, tag="xg", bufs=TGRP)
                    xgv = xg.rearrange("p (c q) -> p c q", c=CB)
                    if tg == 0:
                        nc.sync.dma_start(xgv[:, 0:1], xtr[:, 0:1, 0:QG])
                        nc.sync.dma_start(xgv[:, 1:CB], xtr[:, 1:CB, 0:QG])
                    else:
                        nc.sync.dma_start(
                            xgv, xtr[:, :, tg * QG:(tg + 1) * QG]
                        )
                    kps = ppool.tile([128, QG], F32, tag="kps")
                    for c in range(CB):
                        nc.tensor.matmul(
                            kps,
                            lhsT=wks[:, c * H:(c + 1) * H],
                            rhs=xg[:, c * QG:(c + 1) * QG],
                            start=(c == 0),
                            stop=(c == CB - 1),
                        )
                    nc.any.tensor_copy(KT[:, tg * QG:(tg + 1) * QG], kps)
                    vps = ppool.tile([128, QG], F32, tag="vps")
                    for c in range(CB):
                        nc.tensor.matmul(
                            vps,
                            lhsT=wvs[:, c * H:(c + 1) * H],
                            rhs=xg[:, c * QG:(c + 1) * QG],
                            start=(c == 0),
                            stop=(c == CB - 1),
                        )
                    nc.any.tensor_copy(VT[:, tg * QG:(tg + 1) * QG], vps)
                    for tb in range(QG // 128):
                        t = tg * (QG // 128) + tb
                        tps = ppool.tile([128, 128], BF16, tag="tps")
                        nc.tensor.transpose(
                            tps, VT[:, t * 128:(t + 1) * 128], ident
                        )
                        nc.vector.tensor_copy(
                            VVa[:, t * 65:t * 65 + 64], tps[:, 0:64]
                        )
                        nc.vector.tensor_copy(
                            VVb[:, t * 64:(t + 1) * 64], tps[:, 64:128]
                        )
                for i in range(NG):
                    xq = xinp.tile([128, CB * QG], BF16, tag="xq", bufs=NG)
                    nc.sync.dma_start(
                        xq.rearrange("p (c q) -> p c q", c=CB),
                        xtq.rearrange("(c p) t -> p c t", p=128)[:, :, i * QG:(i + 1) * QG],
                    )
                    qps = ppool.tile([128, QG], F32, tag="qps")
                    for c in range(CB):
                        nc.tensor.matmul(
                            qps,
                            lhsT=wqs[:, c * H:(c + 1) * H],
                            rhs=xq[:, c * QG:(c + 1) * QG],
                            start=(c == 0),
                            stop=(c == CB - 1),
                        )
                    nc.any.tensor_copy(QT[:, i * QG:(i + 1) * QG], qps)

            # --- phase 2: attention ---
            nc.sync.dma_start(
                masks.rearrange("p (m q) -> p m q", m=8),
                msk.rearrange("m p q -> p m q"),
            )
            with tc.tile_pool(name="ap", bufs=2, space="PSUM") as apool:
                for i in range(NG):
                    span = SPANS[i]
                    nchunks = span // 128
                    otpsA = apool.tile([65, QG], F32, tag="otpsA", bufs=1)
                    otpsB = apool.tile([64, QG], F32, tag="otpsB", bufs=1)
                    qg = QT[:, i * QG:(i + 1) * QG]
                    for cp in range(nchunks // 2):
                        c0 = 2 * cp
                        sps = apool.tile([128, 2 * QG], F32, tag="sps", bufs=3)
                        for h in range(2):
                            nc.tensor.matmul(
                                sps[:, h * QG:(h + 1) * QG],
                                lhsT=KT[:, (c0 + h) * 128:(c0 + h + 1) * 128],
                                rhs=qg,
                                start=True,
                                stop=True,
                            )
                        pt = attp.tile([128, 2 * QG], BF16, tag="pt")
                        nc.scalar.activation(
                            pt, sps, mybir.ActivationFunctionType.Exp, scale=SCALE
                        )
                        m = c0 - (nchunks - 8)
                        if m >= 0:
                            nc.vector.tensor_tensor(
                                pt, pt, masks[:, m * QG:(m + 2) * QG],
                                op=AluOpType.mult,
                            )
                        for h in range(2):
                            c = c0 + h
                            ph = pt[:, h * QG:(h + 1) * QG]
                            nc.tensor.matmul(
                                otpsA,
                                lhsT=VVa[:, c * 65:(c + 1) * 65],
                                rhs=ph,
                                start=(c == 0),
                                stop=(c == nchunks - 1),
                            )
                            nc.tensor.matmul(
                                otpsB,
                                lhsT=VVb[:, c * 64:(c + 1) * 64],
                                rhs=ph,
                                start=(c == 0),
                                stop=(c == nchunks - 1),
                            )
                    srow = epip.tile([1, QG], F32, tag="srow")
                    nc.scalar.copy(srow, otpsA[64:65, :])
                    rbb = epip.tile([128, QG], F32, tag="rbb")
                    nc.gpsimd.partition_broadcast(rbb, srow)
                    rbr = epip.tile([128, QG], F32, tag="rbr")
                    nc.vector.reciprocal_approx_fast(rbr, rbb)
                    ot = epip.tile([128, QG], F32, tag="ot")
                    nc.vector.tensor_tensor(
                        ot[0:64, :], otpsA[0:64, :], rbr[0:64, :], op=AluOpType.mult
                    )
                    nc.vector.tensor_tensor(
                        ot[64:128, :], otpsB, rbr[64:128, :], op=AluOpType.mult
                    )
                    nc.sync.dma_start(outT[:, i * QG:(i + 1) * QG], ot)

    if not nc.is_finalized():
        nc.finalize()
    return nc


_NC_CACHE = None


def _get_program():
    global _NC_CACHE
    if _NC_CACHE is None:
        _NC_CACHE = _build_program()
    return _NC_CACHE


def _make_masks(j: int) -> np.ndarray:
    """Mask stack [8, 128, QG] for lane j (f32, 0 or MASKVAL).

    Slot s applies to kv chunk at offset K0 = g - (1024 - j*512) + 128*s
    relative ... concretely: for lane j, the last 8 chunks of each group's
    span get slots 0..7; masked iff global kv > global q, i.e.
    128*(s - 4 + (1 - j) * 4 ... reduces to: kv_i + 128*s - (4 - 4*j)*128 > q_j
    """
    out = np.zeros((8, 128, QG), NPBF16)
    kv = np.arange(128)[:, None]
    q = np.arange(QG)[None, :]
    for s in range(8):
        # lane j: slot s covers the chunk at K0 = g + 128*s - 512*j;
        # multiplicative mask: 0 where kv_global > q_global else 1
        rel = 128 * s - 512 * j
        out[s] = np.where(rel + kv > q, 0.0, 1.0).astype(NPBF16)
    return out


def _run(inputs: dict, trace: bool = False, trace_kwargs: dict | None = None):
    x = np.asarray(inputs["x"], np.float32)
    Wk = np.asarray(inputs["Wk"], np.float32)
    Wq = np.asarray(inputs["Wq"], np.float32)
    Wv = np.asarray(inputs["Wv"], np.float32)

    nc = _get_program()

    wk16 = Wk.astype(NPBF16)
    wq16 = Wq.astype(NPBF16)
    wv16 = Wv.astype(NPBF16)
    msks = [_make_masks(j) for j in range(2)]

    in_maps = []
    for b in range(B):
        xtb = np.ascontiguousarray(x[b].T).astype(NPBF16)  # [C, T]
        for j in range(2):
            xtq = np.concatenate(
                [xtb[:, (2 * i + j) * QG:(2 * i + j + 1) * QG] for i in range(NG)],
                axis=1,
            )
            in_maps.append(
                {
                    "xt": xtb,
                    "xtq": np.ascontiguousarray(xtq),
                    "wk": wk16,
                    "wq": wq16,
                    "wv": wv16,
                    "msk": msks[j],
                }
            )

    res = run_bass_kernel_spmd(
        nc,
        in_maps,
        core_ids=list(range(NCORES)),
        trace=trace,
        **(trace_kwargs or {}),
    )

    out = np.empty((B, T, H), np.float32)
    for core in range(NCORES):
        b, j = divmod(core, 2)
        oT = np.asarray(res.results[core]["outT"], np.float32)  # [H, NG*QG]
        for i in range(NG):
            g = (2 * i + j) * QG
            out[b, g:g + QG, :] = oT[:, i * QG:(i + 1) * QG].T
    return out, res


def kernel(**inputs) -> np.ndarray:
    out, _ = _run(inputs, trace=False)
    return out


# revision 15
# speedup vs baseline: 1.1790x; 1.1790x over previous
"""Bass/Trainium2 kernel for a single-head causal decoder attention head.

Reference computation (fp32):
    k = x @ Wk; q = x @ Wq; v = x @ Wv            # [B,T,H]
    att = softmax(causal(q k^T / sqrt(H)))        # [B,T,T]
    out = att @ v                                 # [B,T,H]
with B=4, T=4096, C=1024, H=128.

Sharding: 8 cores = 4 batches x 2 query-interleave lanes (j in {0,1}).
Core (b, j) handles q-groups [(2i+j)*512, (2i+j+1)*512) for i in 0..3 and
runs a *uniform* kv-span schedule {1024, 2048, 3072, 4096} for groups
0..3, so all 8 cores execute the same instruction stream (SPMD, one NEFF)
while per-core DRAM data (x^T slices, q-column gather, mask stack) makes
the math come out right.  Causality beyond each group's true span is
enforced by additive -30000 masks on the last 8 kv chunks of each group.

Dataflow (per core, transposed land so no on-chip transposes are needed):
    KT [H, T]   = Wk^T x^T        (8 c-chunk matmuls per 512 kv cols)
    QT [H, 2048]= Wq^T xq^T
    V  [kv, H]  (32 blocks)       (lhsT = x^T chunk, rhs = Wv chunk)
    per q-group, per kv chunk c:
        S^T  = KT_c^T QT_g                 (PSUM [128kv, 512q])
        S^T += mask (last 8 chunks, DVE)
        P^T  = exp(S^T / sqrt(H))          (ACT, bf16 -> SBUF)
        outT += V_c^T P^T                  (PSUM [128H, 512q], accumulated)
        sums += ones^T P^T                 (PSUM [1, 512q], accumulated)
    outT / sums -> DRAM  (reciprocal + partition-broadcast + multiply)
"""

import sys

sys.path.insert(0, "/opt/trn_rl_repo")

import numpy as np
import ml_dtypes

import concourse.bass as bass
import concourse.mybir as mybir
import concourse.tile as tile
from concourse import bacc
from concourse.alu_op_type import AluOpType
from concourse.masks import make_identity
from concourse.bass_utils import run_bass_kernel_spmd

B, T, C, H = 4, 4096, 1024, 128
NCORES = 8
QG = 512                      # q-group width
NG = 4                        # q-groups per core
SPANS = [1024, 2048, 3072, 4096]  # uniform kv span per group index
CB = C // 128                 # 8 contraction chunks
TGRP = T // QG                # 8 kv col-groups for projections
SCALE = float(H) ** -0.5
MASKVAL = -30000.0

BF16 = mybir.dt.bfloat16
F32 = mybir.dt.float32
NPBF16 = ml_dtypes.bfloat16


def _build_program():
    nc = bacc.Bacc("TRN2", target_bir_lowering=False, debug=False)

    xt = nc.dram_tensor("xt", [C, T], BF16, kind="ExternalInput").ap()
    xtq = nc.dram_tensor("xtq", [C, NG * QG], BF16, kind="ExternalInput").ap()
    wk = nc.dram_tensor("wk", [C, H], BF16, kind="ExternalInput").ap()
    wq = nc.dram_tensor("wq", [C, H], BF16, kind="ExternalInput").ap()
    wv = nc.dram_tensor("wv", [C, H], BF16, kind="ExternalInput").ap()
    msk = nc.dram_tensor("msk", [8, 128, QG], BF16, kind="ExternalInput").ap()
    outT = nc.dram_tensor("outT", [H, NG * QG], F32, kind="ExternalOutput").ap()

    with tile.TileContext(nc) as tc:
        with (
            tc.tile_pool(name="const", bufs=1) as constp,
            tc.tile_pool(name="kvq", bufs=1) as kvqp,
            tc.tile_pool(name="xin", bufs=2) as xinp,
            tc.tile_pool(name="attb", bufs=4) as attp,
            tc.tile_pool(name="epi", bufs=2) as epip,
        ):
            # --- persistent SBUF tensors ---
            wks = constp.tile([128, CB * H], BF16, tag="wks")
            wqs = constp.tile([128, CB * H], BF16, tag="wqs")
            wvs = constp.tile([128, CB * H], BF16, tag="wvs")
            for eng, ws, w in (
                (nc.scalar, wks, wk), (nc.scalar, wqs, wq), (nc.gpsimd, wvs, wv)
            ):
                eng.dma_start(
                    ws.rearrange("p (c h) -> p c h", c=CB),
                    w.rearrange("(c p) h -> p c h", p=128),
                )
            masks = constp.tile([128, 8 * QG], BF16, tag="masks")
            ident = constp.tile([128, 128], BF16, tag="ident")
            make_identity(nc, ident)

            KT = kvqp.tile([128, T], BF16, tag="KT")
            VT = kvqp.tile([128, T], BF16, tag="VT")
            VV = kvqp.tile([128, (T // 128) * H], BF16, tag="VV")
            QT = kvqp.tile([128, NG * QG], BF16, tag="QT")
            ones = kvqp.tile([128, 128], BF16, tag="ones")
            nc.vector.memset(ones, 1.0)

            # --- phase 1: projections ---
            with tc.tile_pool(name="pp", bufs=2, space="PSUM") as ppool:
                xtr = xt.rearrange("(c p) t -> p c t", p=128)
                for tg in range(TGRP):
                    xg = xinp.tile([128, CB * QG], BF16, tag="xg", bufs=TGRP)
                    xgv = xg.rearrange("p (c q) -> p c q", c=CB)
                    if tg == 0:
                        nc.sync.dma_start(xgv[:, 0:1], xtr[:, 0:1, 0:QG])
                        nc.sync.dma_start(xgv[:, 1:CB], xtr[:, 1:CB, 0:QG])
                    else:
                        nc.sync.dma_start(
                            xgv, xtr[:, :, tg * QG:(tg + 1) * QG]
                        )
                    kps = ppool.tile([128, QG], F32, tag="kps")
                    for c in range(CB):
                        nc.tensor.matmul(
                            kps,
                            lhsT=wks[:, c * H:(c + 1) * H],
                            rhs=xg[:, c * QG:(c + 1) * QG],
                            start=(c == 0),
                            stop=(c == CB - 1),
                        )
                    nc.any.tensor_copy(KT[:, tg * QG:(tg + 1) * QG], kps)
                    vps = ppool.tile([128, QG], F32, tag="vps")
                    for c in range(CB):
                        nc.tensor.matmul(
                            vps,
                            lhsT=wvs[:, c * H:(c + 1) * H],
                            rhs=xg[:, c * QG:(c + 1) * QG],
                            start=(c == 0),
                            stop=(c == CB - 1),
                        )
                    nc.any.tensor_copy(VT[:, tg * QG:(tg + 1) * QG], vps)
                    for tb in range(QG // 128):
                        t = tg * (QG // 128) + tb
                        tps = ppool.tile([128, 128], BF16, tag="tps")
                        nc.tensor.transpose(
                            tps, VT[:, t * 128:(t + 1) * 128], ident
                        )
                        nc.vector.tensor_copy(VV[:, t * H:(t + 1) * H], tps)
                for i in range(NG):
                    xq = xinp.tile([128, CB * QG], BF16, tag="xq", bufs=NG)
                    nc.sync.dma_start(
                        xq.rearrange("p (c q) -> p c q", c=CB),
                        xtq.rearrange("(c p) t -> p c t", p=128)[:, :, i * QG:(i + 1) * QG],
                    )
                    qps = ppool.tile([128, QG], F32, tag="qps")
                    for c in range(CB):
                        nc.tensor.matmul(
                            qps,
                            lhsT=wqs[:, c * H:(c + 1) * H],
                            rhs=xq[:, c * QG:(c + 1) * QG],
                            start=(c == 0),
                            stop=(c == CB - 1),
                        )
                    nc.any.tensor_copy(QT[:, i * QG:(i + 1) * QG], qps)

            # --- phase 2: attention ---
            nc.sync.dma_start(
                masks.rearrange("p (m q) -> p m q", m=8),
                msk.rearrange("m p q -> p m q"),
            )
            with tc.tile_pool(name="ap", bufs=2, space="PSUM") as apool:
                for i in range(NG):
                    span = SPANS[i]
                    nchunks = span // 128
                    otps = apool.tile([128, QG], F32, tag="otps", bufs=1)
                    smps = apool.tile([128, QG], F32, tag="smps", bufs=1)
                    qg = QT[:, i * QG:(i + 1) * QG]
                    for cp in range(nchunks // 2):
                        c0 = 2 * cp
                        sps = apool.tile([128, 2 * QG], F32, tag="sps", bufs=3)
                        for h in range(2):
                            nc.tensor.matmul(
                                sps[:, h * QG:(h + 1) * QG],
                                lhsT=KT[:, (c0 + h) * 128:(c0 + h + 1) * 128],
                                rhs=qg,
                                start=True,
                                stop=True,
                            )
                        pt = attp.tile([128, 2 * QG], BF16, tag="pt")
                        nc.scalar.activation(
                            pt, sps, mybir.ActivationFunctionType.Exp, scale=SCALE
                        )
                        m = c0 - (nchunks - 8)
                        if m >= 0:
                            nc.vector.tensor_tensor(
                                pt, pt, masks[:, m * QG:(m + 2) * QG],
                                op=AluOpType.mult,
                            )
                        for h in range(2):
                            c = c0 + h
                            ph = pt[:, h * QG:(h + 1) * QG]
                            nc.tensor.matmul(
                                otps,
                                lhsT=VV[:, c * H:(c + 1) * H],
                                rhs=ph,
                                start=(c == 0),
                                stop=(c == nchunks - 1),
                            )
                            nc.tensor.matmul(
                                smps,
                                lhsT=ones,
                                rhs=ph,
                                start=(c == 0),
                                stop=(c == nchunks - 1),
                            )
                    rb = epip.tile([128, QG], F32, tag="rb")
                    nc.vector.reciprocal_approx_fast(rb, smps)
                    ot = epip.tile([128, QG], F32, tag="ot")
                    nc.vector.tensor_tensor(ot, otps, rb, op=AluOpType.mult)
                    nc.sync.dma_start(outT[:, i * QG:(i + 1) * QG], ot)

    if not nc.is_finalized():
        nc.finalize()
    return nc


_NC_CACHE = None


def _get_program():
    global _NC_CACHE
    if _NC_CACHE is None:
        _NC_CACHE = _build_program()
    return _NC_CACHE


def _make_masks(j: int) -> np.ndarray:
    """Mask stack [8, 128, QG] for lane j (f32, 0 or MASKVAL).

    Slot s applies to kv chunk at offset K0 = g - (1024 - j*512) + 128*s
    relative ... concretely: for lane j, the last 8 chunks of each group's
    span get slots 0..7; masked iff global kv > global q, i.e.
    128*(s - 4 + (1 - j) * 4 ... reduces to: kv_i + 128*s - (4 - 4*j)*128 > q_j
    """
    out = np.zeros((8, 128, QG), NPBF16)
    kv = np.arange(128)[:, None]
    q = np.arange(QG)[None, :]
    for s in range(8):
        # lane j: slot s covers the chunk at K0 = g + 128*s - 512*j;
        # multiplicative mask: 0 where kv_global > q_global else 1
        rel = 128 * s - 512 * j
        out[s] = np.where(rel + kv > q, 0.0, 1.0).astype(NPBF16)
    return out


def _run(inputs: dict, trace: bool = False, trace_kwargs: dict | None = None):
    x = np.asarray(inputs["x"], np.float32)
    Wk = np.asarray(inputs["Wk"], np.float32)
    Wq = np.asarray(inputs["Wq"], np.float32)
    Wv = np.asarray(inputs["Wv"], np.float32)

    nc = _get_program()

    wk16 = Wk.astype(NPBF16)
    wq16 = Wq.astype(NPBF16)
    wv16 = Wv.astype(NPBF16)
    msks = [_make_masks(j) for j in range(2)]

    in_maps = []
    for b in range(B):
        xtb = np.ascontiguousarray(x[b].T).astype(NPBF16)  # [C, T]
        for j in range(2):
            xtq = np.concatenate(
                [xtb[:, (2 * i + j) * QG:(2 * i + j + 1) * QG] for i in range(NG)],
                axis=1,
            )
            in_maps.append(
                {
                    "xt": xtb,
                    "xtq": np.ascontiguousarray(xtq),
                    "wk": wk16,
                    "wq": wq16,
                    "wv": wv16,
                    "msk": msks[j],
                }
            )

    res = run_bass_kernel_spmd(
        nc,
        in_maps,
        core_ids=list(range(NCORES)),
        trace=trace,
        **(trace_kwargs or {}),
    )

    out = np.empty((B, T, H), np.float32)
    for core in range(NCORES):
        b, j = divmod(core, 2)
        oT = np.asarray(res.results[core]["outT"], np.float32)  # [H, NG*QG]
        for i in range(NG):
            g = (2 * i + j) * QG
            out[b, g:g + QG, :] = oT[:, i * QG:(i + 1) * QG].T
    return out, res


def kernel(**inputs) -> np.ndarray:
    out, _ = _run(inputs, trace=False)
    return out
